# revision 6
# baseline (speedup 1.0000x reference)
"""Multi-head attention (B=4, S=2048, D=1024, H=16) on 8 trn2 NeuronCores.

Sharding: core = (batch b, head-group g) with b = core//2, g = core%2.
Each core handles one batch and 8 heads (512 of the 1024 d_model dims):
  - host pre-transposes query/key/value[b] -> [1024, 2048] so the device
    never transposes activations (and pre-casts to the matmul dtype)
  - device computes Q^T, K^T (head dims on partitions) and V (natural),
    attention with *transposed* scores S^T = K_h @ Q_h^T so softmax's
    denominator comes out of the PV matmul via a ones-column appended to V
  - output projection vs Wo[g*512:(g+1)*512, :] gives a partial [2048,1024]
  - host sums the two group partials per batch and adds bv@Wo + bo
Matmul operand dtype is MM_DT (bf16 default: full-rate PE streaming + FWL;
f32r fallback: fp22 multiplies at half stream rate). PSUM accumulation and
the softmax normalization chain stay fp32.

v2 scheduling: parallel prologue DMAs across engine queues (sync/scalar/
vector/gpsimd) so the first exp fires ~16us in instead of ~49us; filler
groups paced into fixed kt slots that leave each (qc,hh) loop's tail clear
so the next loop's scores matmuls aren't queued behind leftover projection
work; per-loop PV evict + normalize emitted immediately after each head's
kt sweep.
"""

import os
import numpy as np
from contextlib import ExitStack

B = 4
S = 2048
D = 1024
H = 16
DK = 64
NCORES = 8
GH = 8          # heads per core (group)
GD = GH * DK    # 512 head dims per core
NCH = GD // 128  # 4 chunks of 128 output dims
KT = S // 128    # 16 key tiles
QC = 1024        # q chunk width for attention
NQC = S // QC    # 2
SC = 512         # s chunk width for projections
NSC = S // SC    # 8
DMT = D // 128   # 8 d_model tiles

MM_DT = os.environ.get("MM_DT", "bf16")  # "bf16" | "f32r"

_CACHE = {}


def _np_mm_dtype():
    if MM_DT == "bf16":
        import ml_dtypes
        return ml_dtypes.bfloat16
    return np.float32


def _build_program():
    import concourse.mybir as mybir
    import concourse.tile as tile
    from concourse import bacc

    f32 = mybir.dt.float32
    dmm = mybir.dt.bfloat16 if MM_DT == "bf16" else mybir.dt.float32r

    nc = bacc.Bacc("TRN2", target_bir_lowering=False, debug=False,
                   num_devices=NCORES)

    xqT = nc.dram_tensor("xqT", [D, S], dmm, kind="ExternalInput").ap()
    xkT = nc.dram_tensor("xkT", [D, S], dmm, kind="ExternalInput").ap()
    xvT = nc.dram_tensor("xvT", [D, S], dmm, kind="ExternalInput").ap()
    wq = nc.dram_tensor("wq", [D, GD], dmm, kind="ExternalInput").ap()
    wk = nc.dram_tensor("wk", [D, GD], dmm, kind="ExternalInput").ap()
    wv = nc.dram_tensor("wv", [D, GD], dmm, kind="ExternalInput").ap()
    wo = nc.dram_tensor("wo", [GD, D], dmm, kind="ExternalInput").ap()
    bq = nc.dram_tensor("bq", [GD], f32, kind="ExternalInput").ap()
    bk = nc.dram_tensor("bk", [GD], f32, kind="ExternalInput").ap()
    out = nc.dram_tensor("out", [S, D], f32, kind="ExternalOutput").ap()

    Exp = mybir.ActivationFunctionType.Exp

    with tile.TileContext(nc) as tc, ExitStack() as ctx:
        # ---- pools (slots are statically reserved per tag) ----
        p_qt = ctx.enter_context(tc.tile_pool(name="qt", bufs=GH))
        p_kt = ctx.enter_context(tc.tile_pool(name="kt", bufs=GH))
        p_v = ctx.enter_context(tc.tile_pool(name="v", bufs=KT))
        p_ot = ctx.enter_context(tc.tile_pool(name="ot", bufs=NCH))
        p_wvo = ctx.enter_context(tc.tile_pool(name="wvo", bufs=1))
        p_wc = ctx.enter_context(tc.tile_pool(name="wc", bufs=1))
        p_bias = ctx.enter_context(tc.tile_pool(name="bias", bufs=1))
        p_xs = ctx.enter_context(tc.tile_pool(name="xs", bufs=4))
        p_pt = ctx.enter_context(tc.tile_pool(name="pt", bufs=7))
        p_zr = ctx.enter_context(tc.tile_pool(name="zr", bufs=2))
        p_rb = ctx.enter_context(tc.tile_pool(name="rb", bufs=2))
        p_st = ctx.enter_context(tc.tile_pool(name="st", bufs=2))
        p_ov = ctx.enter_context(tc.tile_pool(name="ov", bufs=2))
        # PSUM: ps 3 slots x 2 banks + pv 1 slot x 2 banks = all 8 banks
        p_ps = ctx.enter_context(tc.tile_pool(name="ps", bufs=3, space="PSUM"))
        p_pv = ctx.enter_context(tc.tile_pool(name="pv", bufs=1, space="PSUM"))

        # ---- biases + ones (tiny, sync queue) ----
        bq_sb = p_bias.tile([128, NCH], f32, tag="bq")
        nc.sync.dma_start(out=bq_sb[:], in_=bq.rearrange("(a p) -> p a", p=128))
        bk_sb = p_bias.tile([128, NCH], f32, tag="bk")
        nc.sync.dma_start(out=bk_sb[:], in_=bk.rearrange("(a p) -> p a", p=128))
        ones_sb = p_bias.tile([128, 1], f32, tag="ones")
        nc.vector.memset(ones_sb[:], 1.0)

        # ---- weight DMAs on gpsimd so sync/scalar carry the xs slices ----
        wk_sb = p_wc.tile([128, DMT, GD], dmm, tag="wkc", name="wk_sb")
        nc.gpsimd.dma_start(out=wk_sb[:],
                            in_=wk.rearrange("(a p) d -> p a d", p=128))
        wq_sb = p_wc.tile([128, DMT, GD], dmm, tag="wqc", name="wq_sb")
        nc.gpsimd.dma_start(out=wq_sb[:],
                            in_=wq.rearrange("(a p) d -> p a d", p=128))

        qt_sb = [None] * GH
        kt_sb = [None] * GH
        ot_sb = [None] * NCH
        v_sb = []
        vstate = {}

        # ---- V projection: V_sb[st] = [128 s, GH, 65] (col 64 = ones) ----
        def emit_v_st(st):
            if st % 4 == 0:
                xv_t = p_xs.tile([128, DMT, 512], dmm, tag="xs",
                                 name=f"xv{st}")
                # scalar queue only for the prologue chunk; later chunks go
                # on gpsimd so they never stall the exp stream on ACT
                eng = nc.scalar if st == 0 else nc.gpsimd
                eng.dma_start(
                    out=xv_t[:],
                    in_=xvT[:, st * 128:st * 128 + 512].rearrange(
                        "(a p) s -> p a s", p=128),
                )
                vstate["xv"] = xv_t
            xv_t = vstate["xv"]
            sub = (st % 4) * 128
            ps = p_ps.tile([128, 1024], f32, tag="ps", name=f"psv{st}")
            for a in range(DMT):
                nc.tensor.matmul(
                    out=ps[:, 0:GD],
                    lhsT=xv_t[:, a, sub:sub + 128],
                    rhs=wv_sb[:, a, :],
                    start=(a == 0), stop=(a == DMT - 1),
                )
            vt = p_v.tile([128, GH, 65], dmm, tag="v", name=f"v{st}")
            nc.vector.tensor_copy(
                out=vt[:, :, 0:DK],
                in_=ps[:, 0:GD].rearrange("p (h d) -> p h d", h=GH),
            )
            nc.vector.tensor_copy(
                out=vt[:, :, DK:65],
                in_=ones_sb.unsqueeze(1).broadcast_to([128, GH, 1]))
            v_sb.append(vt)

        def v_filler(st):
            def emit():
                emit_v_st(st)
            return emit

        # ---- Q/K projections for head pair c; one group = one s-chunk ----
        def alloc_pair(c):
            for hh in range(2):
                hg = 2 * c + hh
                qt_sb[hg] = p_qt.tile([128, S], dmm, tag="qt", name=f"qt{hg}")
                kt_sb[hg] = p_kt.tile([128, S], dmm, tag="kt", name=f"kt{hg}")

        def proj_group(c, which, sc, dma=None):
            """Q^T/K^T slice for heads 2c,2c+1 over s-range sc.
            Head tile [128, S] holds its 64 dims twice (rows 0-63 and
            64-127) so consecutive kt score matmuls alternate PE row
            groups and run concurrently."""
            src, wsb, bsb = ((xqT, wq_sb, bq_sb) if which == "q"
                            else (xkT, wk_sb, bk_sb))
            dsts = ([qt_sb[2 * c], qt_sb[2 * c + 1]] if which == "q"
                    else [kt_sb[2 * c], kt_sb[2 * c + 1]])
            eng = dma if dma is not None else nc.sync

            def emit():
                xs = p_xs.tile([128, DMT, SC], dmm, tag="xs",
                               name=f"xs{which}{c}_{sc}")
                eng.dma_start(
                    out=xs[:],
                    in_=src[:, sc * SC:(sc + 1) * SC].rearrange(
                        "(a p) s -> p a s", p=128),
                )
                ps = p_ps.tile([128, 1024], f32, tag="ps",
                               name=f"psp{which}{c}_{sc}")
                for a in range(DMT):
                    nc.tensor.matmul(
                        out=ps[:, 0:SC],
                        lhsT=wsb[:, a, c * 128:(c + 1) * 128],
                        rhs=xs[:, a, :],
                        start=(a == 0), stop=(a == DMT - 1),
                    )
                s0, s1 = sc * SC, (sc + 1) * SC
                # head 2c native rows 0-63; head 2c+1 native rows 64-127
                nc.vector.tensor_scalar_add(
                    out=dsts[0][0:DK, s0:s1], in0=ps[0:DK, 0:SC],
                    scalar1=bsb[0:DK, c:c + 1])
                nc.vector.tensor_scalar_add(
                    out=dsts[1][DK:128, s0:s1], in0=ps[DK:128, 0:SC],
                    scalar1=bsb[DK:128, c:c + 1])
                # duplicate this slice into the other half right away
                # (SBUF->SBUF DMA) so scores kt for this s-range unblock
                nc.sync.dma_start(out=dsts[0][DK:128, s0:s1],
                                  in_=dsts[0][0:DK, s0:s1])
                nc.sync.dma_start(out=dsts[1][0:DK, s0:s1],
                                  in_=dsts[1][DK:128, s0:s1])
            return emit

        # ---- one attention loop: head hg = 2c+hh, q columns qc*QC.. ----
        def attention_loop(c, qc, hh, slots=(), fillers=None):
            slots = set(slots)
            fillers = fillers if fillers is not None else []
            hg = 2 * c + hh
            if ot_sb[c] is None:
                ot_sb[c] = p_ot.tile([128, S], dmm, tag="ot", name=f"ot{c}")
            pv_ps = p_pv.tile([65, QC], f32, tag="pv",
                              name=f"pv{c}_{qc}_{hh}")
            for kt_i in range(KT):
                if kt_i in slots and fillers:
                    fillers.pop(0)()
                rg = DK * (kt_i % 2)
                ps = p_ps.tile([128, QC], f32, tag="ps",
                               name=f"pss{c}_{qc}_{kt_i}_{hh}")
                for half in range(QC // 512):
                    q0 = qc * QC + half * 512
                    nc.tensor.matmul(
                        out=ps[:, half * 512:(half + 1) * 512],
                        lhsT=kt_sb[hg][rg:rg + DK,
                                       kt_i * 128:(kt_i + 1) * 128],
                        rhs=qt_sb[hg][rg:rg + DK, q0:q0 + 512],
                        start=True, stop=True,
                    )
                pt = p_pt.tile([128, QC], dmm, tag="pt",
                               name=f"pt{c}_{qc}_{kt_i}_{hh}")
                nc.scalar.activation(pt[:], ps[:], Exp, bias=0.0, scale=0.125)
                for half in range(QC // 512):
                    nc.tensor.matmul(
                        out=pv_ps[:, half * 512:(half + 1) * 512],
                        lhsT=v_sb[kt_i][:, hg, :],
                        rhs=pt[:, half * 512:(half + 1) * 512],
                        start=(kt_i == 0), stop=(kt_i == KT - 1),
                    )
            # evict PV psum right away to release its bank pair
            ovt = p_ov.tile([65, QC], f32, tag="ov", name=f"ov{c}_{qc}_{hh}")
            nc.vector.tensor_copy(out=ovt[:], in_=pv_ps[:])
            # normalize off the critical path:
            # O^T = PV[0:64] * broadcast(1 / PV[64])
            zs = p_zr.tile([DK, QC // DK], f32, tag="zs",
                           name=f"zs{c}_{qc}_{hh}")
            nc.sync.dma_start(out=zs[:], in_=ovt[DK:DK + 1, :])
            nc.vector.reciprocal(out=zs[:], in_=zs[:])
            zr = p_zr.tile([1, QC], f32, tag="zr", name=f"zr{c}_{qc}_{hh}")
            nc.sync.dma_start(out=zr[:], in_=zs[:])
            rb = p_rb.tile([DK, QC], f32, tag="rb", name=f"rb{c}_{qc}_{hh}")
            nc.gpsimd.partition_broadcast(rb[:], zr[:], channels=DK)
            if hh == 0:
                nc.vector.tensor_mul(
                    out=ot_sb[c][0:DK, qc * QC:(qc + 1) * QC],
                    in0=ovt[0:DK, :], in1=rb[:])
            else:
                tmp = p_rb.tile([DK, QC], dmm, tag="rb", name=f"tmp{c}_{qc}")
                nc.vector.tensor_mul(out=tmp[:], in0=ovt[0:DK, :], in1=rb[:])
                nc.sync.dma_start(
                    out=ot_sb[c][DK:128, qc * QC:(qc + 1) * QC],
                    in_=tmp[:])

        # ---- output projection for one 128-row q tile ----
        def emit_final(qt_i):
            ps = p_ps.tile([128, 1024], f32, tag="ps", name=f"pso{qt_i}")
            for cc in range(NCH):
                for half in range(2):
                    nc.tensor.matmul(
                        out=ps[:, half * 512:(half + 1) * 512],
                        lhsT=ot_sb[cc][:, qt_i * 128:(qt_i + 1) * 128],
                        rhs=wo_sb[:, cc, half * 512:(half + 1) * 512],
                        start=(cc == 0), stop=(cc == NCH - 1),
                    )
            st = p_st.tile([128, D], f32, tag="st", name=f"st{qt_i}")
            nc.vector.tensor_copy(out=st[:], in_=ps[:])
            nc.sync.dma_start(out=out[qt_i * 128:(qt_i + 1) * 128, :],
                              in_=st[:])

        def fin_filler(qt_i):
            def emit():
                emit_final(qt_i)
            return emit

        # ================= emission =================
        # prologue: pair-0 Q/K s-chunk 0/1 + V chunk 0, DMAs on parallel
        # queues (k0 sync, q0 vector behind wq, q1 gpsimd behind wk,
        # xv0+wv scalar) -> first scores matmul ~15us in
        alloc_pair(0)
        proj_group(0, "k", 0, dma=nc.sync)()
        proj_group(0, "q", 0, dma=nc.sync)()
        proj_group(0, "q", 1, dma=nc.scalar)()
        wv_sb = p_wvo.tile([128, DMT, GD], dmm, tag="wvo", name="wv_sb")
        nc.scalar.dma_start(out=wv_sb[:],
                            in_=wv.rearrange("(a p) d -> p a d", p=128))
        for st in range(4):
            emit_v_st(st)
        wo_sb = p_wvo.tile([128, NCH, D], dmm, tag="wo", name="wo_sb")
        nc.scalar.dma_start(out=wo_sb[:],
                            in_=wo.rearrange("(a p) n -> p a n", p=128))

        g = lambda c, w, sc: proj_group(c, w, sc)
        alloc_pair(1)
        alloc_pair(2)
        alloc_pair(3)

        # loop 0 consumes v4..v15 + k1..k3 just-in-time (one per kt);
        # later loops take 2-4 groups each in early/mid-kt slots only so
        # loop tails stay clear for the next loop's scores
        sched = {
            (0, 0, 0): (range(0, 15),
                        [v_filler(4), g(0, "k", 1), v_filler(5),
                         g(0, "k", 2), v_filler(6), v_filler(7),
                         g(0, "k", 3), v_filler(8), v_filler(9),
                         v_filler(10), v_filler(11), v_filler(12),
                         v_filler(13), v_filler(14), v_filler(15)]),
            (0, 0, 1): ((1, 4, 7, 10),
                        [g(0, "q", 2), g(0, "q", 3),
                         g(1, "k", 0), g(1, "q", 0)]),
            (0, 1, 0): ((1, 4, 7, 10),
                        [g(1, "q", 1), g(1, "k", 1),
                         g(1, "k", 2), g(1, "q", 2)]),
            (0, 1, 1): ((1, 4, 7), [g(1, "k", 3), g(1, "q", 3)]),
            (1, 0, 0): ((2, 6), [g(2, "k", 0), g(2, "q", 0)]),
            (1, 0, 1): ((2, 6), [g(2, "q", 1), g(2, "k", 1)]),
            (1, 1, 0): ((2, 6), [g(2, "k", 2), g(2, "q", 2)]),
            (1, 1, 1): ((2, 6), [g(2, "k", 3), g(2, "q", 3)]),
            (2, 0, 0): ((2, 6), [g(3, "k", 0), g(3, "q", 0)]),
            (2, 0, 1): ((2, 6), [g(3, "q", 1), g(3, "k", 1)]),
            (2, 1, 0): ((2, 6), [g(3, "k", 2), g(3, "q", 2)]),
            (2, 1, 1): ((2, 6), [g(3, "k", 3), g(3, "q", 3)]),
            (3, 0, 0): ((), []),
            (3, 0, 1): ((), []),
            (3, 1, 0): ((2, 5, 8, 11),
                        [fin_filler(0), fin_filler(1), fin_filler(2),
                         fin_filler(3)]),
            (3, 1, 1): ((2, 5, 8, 11),
                        [fin_filler(4), fin_filler(5), fin_filler(6),
                         fin_filler(7)]),
        }
        for c in range(NCH):
            for qc in range(NQC):
                for hh in range(2):
                    slots, fillers = sched[(c, qc, hh)]
                    attention_loop(c, qc, hh, slots=slots, fillers=fillers)
        for qt_i in range(8, KT):
            emit_final(qt_i)

    nc.compile()
    return nc


def get_program():
    if "nc" not in _CACHE:
        _CACHE["nc"] = _build_program()
    return _CACHE["nc"]


def make_in_maps(inputs):
    dt = _np_mm_dtype()
    q = np.asarray(inputs["query"], np.float32)
    k = np.asarray(inputs["key"], np.float32)
    v = np.asarray(inputs["value"], np.float32)
    Wq = np.asarray(inputs["Wq"], np.float32)
    Wk = np.asarray(inputs["Wk"], np.float32)
    Wv = np.asarray(inputs["Wv"], np.float32)
    Wo = np.asarray(inputs["Wo"], np.float32)
    bq = np.asarray(inputs["bq"], np.float32)
    bk = np.asarray(inputs["bk"], np.float32)
    in_maps = []
    for core in range(NCORES):
        b, g = core // 2, core % 2
        sl = slice(g * GD, (g + 1) * GD)
        in_maps.append({
            "xqT": np.ascontiguousarray(q[b].T).astype(dt),
            "xkT": np.ascontiguousarray(k[b].T).astype(dt),
            "xvT": np.ascontiguousarray(v[b].T).astype(dt),
            "wq": np.ascontiguousarray(Wq[:, sl]).astype(dt),
            "wk": np.ascontiguousarray(Wk[:, sl]).astype(dt),
            "wv": np.ascontiguousarray(Wv[:, sl]).astype(dt),
            "wo": np.ascontiguousarray(Wo[sl, :]).astype(dt),
            "bq": np.ascontiguousarray(bq[sl]),
            "bk": np.ascontiguousarray(bk[sl]),
        })
    return in_maps


def combine_outputs(results, inputs):
    Wo = np.asarray(inputs["Wo"], np.float32)
    bv = np.asarray(inputs["bv"], np.float32)
    bo = np.asarray(inputs["bo"], np.float32)
    out = np.empty((B, S, D), np.float32)
    for b in range(B):
        out[b] = results[2 * b]["out"] + results[2 * b + 1]["out"]
    out += bv @ Wo + bo
    return out


def kernel(**inputs):
    from concourse.bass_utils import run_bass_kernel_spmd
    nc = get_program()
    in_maps = make_in_maps(inputs)
    res = run_bass_kernel_spmd(nc, in_maps, list(range(NCORES)))
    return combine_outputs(res.results, inputs)


# revision 8
# speedup vs baseline: 1.0617x; 1.0617x over previous
"""Multi-head attention (B=4, S=2048, D=1024, H=16) on 8 trn2 NeuronCores.

Sharding: core = (batch b, head-group g) with b = core//2, g = core%2.
Each core handles one batch and 8 heads (512 of the 1024 d_model dims):
  - host pre-tiles query/key/value[b] and the weight slices into the exact
    SBUF layouts ([sc, 128, a, s] slabs / [128, a, d] weights) so every DMA
    is a contiguous 8KB-per-partition transfer (no strided descriptors)
  - device computes Q^T, K^T (head dims on partitions) and V (natural),
    attention with *transposed* scores S^T = K_h @ Q_h^T so softmax's
    denominator comes out of the PV matmul via a ones-column appended to V
  - output projection vs Wo[g*512:(g+1)*512, :] gives a partial [2048,1024]
  - host sums the two group partials per batch and adds bv@Wo + bo
Matmul operand dtype is MM_DT (bf16 default). PSUM accumulation and the
softmax normalization chain stay fp32.

v3 scheduling: projection/output-projection work is fed to the PE in 4-MM
chunks from a global filler queue, one chunk per odd kt slot, which fits
the ~1us/2kt PE slack under the ACT-bound exp stream (whole 8-MM groups
stalled the exp pipeline ~2.1us each). Prologue DMAs are ordered per queue
(gpsimd: wk,wq,wv / sync: xs k0,q0,q1 / scalar: xv0,wo) so the first
scores inputs arrive first.
"""

import os
import numpy as np
from contextlib import ExitStack

B = 4
S = 2048
D = 1024
H = 16
DK = 64
NCORES = 8
GH = 8          # heads per core (group)
GD = GH * DK    # 512 head dims per core
NCH = GD // 128  # 4 chunks of 128 output dims
KT = S // 128    # 16 key tiles
QC = 1024        # q chunk width for attention
NQC = S // QC    # 2
SC = 512         # s chunk width for projections
NSC = S // SC    # 4
DMT = D // 128   # 8 d_model tiles

MM_DT = os.environ.get("MM_DT", "bf16")  # "bf16" | "f32r"

_CACHE = {}


def _np_mm_dtype():
    if MM_DT == "bf16":
        import ml_dtypes
        return ml_dtypes.bfloat16
    return np.float32


def _build_program():
    import concourse.mybir as mybir
    import concourse.tile as tile
    from concourse import bacc

    f32 = mybir.dt.float32
    dmm = mybir.dt.bfloat16 if MM_DT == "bf16" else mybir.dt.float32r

    nc = bacc.Bacc("TRN2", target_bir_lowering=False, debug=False,
                   num_devices=NCORES)

    xq = nc.dram_tensor("xq", [NSC, 128, DMT, SC], dmm,
                        kind="ExternalInput").ap()
    xk = nc.dram_tensor("xk", [NSC, 128, DMT, SC], dmm,
                        kind="ExternalInput").ap()
    xv = nc.dram_tensor("xv", [NSC, 128, DMT, SC], dmm,
                        kind="ExternalInput").ap()
    wq = nc.dram_tensor("wq", [128, DMT, GD], dmm, kind="ExternalInput").ap()
    wk = nc.dram_tensor("wk", [128, DMT, GD], dmm, kind="ExternalInput").ap()
    wv = nc.dram_tensor("wv", [128, DMT, GD], dmm, kind="ExternalInput").ap()
    wo = nc.dram_tensor("wo", [128, NCH, D], dmm, kind="ExternalInput").ap()
    bq = nc.dram_tensor("bq", [128, NCH], f32, kind="ExternalInput").ap()
    bk = nc.dram_tensor("bk", [128, NCH], f32, kind="ExternalInput").ap()
    out = nc.dram_tensor("out", [S, D], f32, kind="ExternalOutput").ap()

    Exp = mybir.ActivationFunctionType.Exp

    with tile.TileContext(nc) as tc, ExitStack() as ctx:
        # ---- pools (slots are statically reserved per tag) ----
        p_qt = ctx.enter_context(tc.tile_pool(name="qt", bufs=GH))
        p_kt = ctx.enter_context(tc.tile_pool(name="kt", bufs=GH))
        p_v = ctx.enter_context(tc.tile_pool(name="v", bufs=KT))
        p_ot = ctx.enter_context(tc.tile_pool(name="ot", bufs=NCH))
        p_wvo = ctx.enter_context(tc.tile_pool(name="wvo", bufs=1))
        p_wc = ctx.enter_context(tc.tile_pool(name="wc", bufs=1))
        p_bias = ctx.enter_context(tc.tile_pool(name="bias", bufs=1))
        p_xs = ctx.enter_context(tc.tile_pool(name="xs", bufs=4))
        p_pt = ctx.enter_context(tc.tile_pool(name="pt", bufs=7))
        p_zr = ctx.enter_context(tc.tile_pool(name="zr", bufs=2))
        p_rb = ctx.enter_context(tc.tile_pool(name="rb", bufs=2))
        p_st = ctx.enter_context(tc.tile_pool(name="st", bufs=2))
        p_ov = ctx.enter_context(tc.tile_pool(name="ov", bufs=2))
        # PSUM: ps 3 slots x 2 banks + pv 1 slot x 2 banks = all 8 banks
        p_ps = ctx.enter_context(tc.tile_pool(name="ps", bufs=3, space="PSUM"))
        p_pv = ctx.enter_context(tc.tile_pool(name="pv", bufs=1, space="PSUM"))

        # ---- biases + ones (tiny, sync queue) ----
        bq_sb = p_bias.tile([128, NCH], f32, tag="bq")
        nc.sync.dma_start(out=bq_sb[:], in_=bq)
        bk_sb = p_bias.tile([128, NCH], f32, tag="bk")
        nc.sync.dma_start(out=bk_sb[:], in_=bk)
        ones_sb = p_bias.tile([128, 1], f32, tag="ones")
        nc.vector.memset(ones_sb[:], 1.0)

        # ---- weight DMAs on gpsimd, ordered by first use ----
        wk_sb = p_wc.tile([128, DMT, GD], dmm, tag="wkc", name="wk_sb")
        nc.gpsimd.dma_start(out=wk_sb[:], in_=wk)
        wq_sb = p_wc.tile([128, DMT, GD], dmm, tag="wqc", name="wq_sb")
        nc.gpsimd.dma_start(out=wq_sb[:], in_=wq)
        wv_sb = p_wvo.tile([128, DMT, GD], dmm, tag="wvo", name="wv_sb")
        nc.gpsimd.dma_start(out=wv_sb[:], in_=wv)

        qt_sb = [None] * GH
        kt_sb = [None] * GH
        ot_sb = [None] * NCH
        v_sb = []
        vstate = {}

        # ---- V projection: V_sb[st] = [128 s, GH, 65] (col 64 = ones) ----
        def emit_v_st(st):
            if st % 4 == 0:
                xv_t = p_xs.tile([128, DMT, 512], dmm, tag="xs",
                                 name=f"xv{st}")
                # scalar queue for the prologue chunk; later chunks go on
                # sync so they never stall the exp stream on ACT
                eng = nc.scalar if st == 0 else nc.sync
                eng.dma_start(out=xv_t[:], in_=xv[st // 4])
                vstate["xv"] = xv_t
            xv_t = vstate["xv"]
            sub = (st % 4) * 128
            ps = p_ps.tile([128, 1024], f32, tag="ps", name=f"psv{st}")
            for a in range(DMT):
                nc.tensor.matmul(
                    out=ps[:, 0:GD],
                    lhsT=xv_t[:, a, sub:sub + 128],
                    rhs=wv_sb[:, a, :],
                    start=(a == 0), stop=(a == DMT - 1),
                )
            vt = p_v.tile([128, GH, 65], dmm, tag="v", name=f"v{st}")
            nc.vector.tensor_copy(
                out=vt[:, :, 0:DK],
                in_=ps[:, 0:GD].rearrange("p (h d) -> p h d", h=GH),
            )
            nc.vector.tensor_copy(
                out=vt[:, :, DK:65],
                in_=ones_sb.unsqueeze(1).broadcast_to([128, GH, 1]))
            v_sb.append(vt)

        def v_filler(st):
            def emit():
                emit_v_st(st)
            return emit

        # ---- Q/K projections for head pair c; one group = one s-chunk ----
        def alloc_pair(c):
            for hh in range(2):
                hg = 2 * c + hh
                qt_sb[hg] = p_qt.tile([128, S], dmm, tag="qt", name=f"qt{hg}")
                kt_sb[hg] = p_kt.tile([128, S], dmm, tag="kt", name=f"kt{hg}")

        def proj_chunks(c, which, sc, dma=None):
            """Q^T/K^T slice for heads 2c,2c+1 over s-range sc, split into
            two 4-MM chunks so filler work interleaves with the attention
            matmul stream without starving ACT. Head tile [128, S] holds
            its 64 dims twice (rows 0-63 and 64-127) so consecutive kt
            score matmuls alternate PE row groups and run concurrently."""
            src, wsb, bsb = ((xq, wq_sb, bq_sb) if which == "q"
                            else (xk, wk_sb, bk_sb))
            dsts = ([qt_sb[2 * c], qt_sb[2 * c + 1]] if which == "q"
                    else [kt_sb[2 * c], kt_sb[2 * c + 1]])
            eng = dma if dma is not None else nc.sync
            state = {}

            def emit_a():
                xs = p_xs.tile([128, DMT, SC], dmm, tag="xs",
                               name=f"xs{which}{c}_{sc}")
                eng.dma_start(out=xs[:], in_=src[sc])
                ps = p_ps.tile([128, 1024], f32, tag="ps",
                               name=f"psp{which}{c}_{sc}")
                for a in range(4):
                    nc.tensor.matmul(
                        out=ps[:, 0:SC],
                        lhsT=wsb[:, a, c * 128:(c + 1) * 128],
                        rhs=xs[:, a, :],
                        start=(a == 0), stop=False,
                    )
                state["xs"], state["ps"] = xs, ps

            def emit_b():
                xs, ps = state["xs"], state["ps"]
                for a in range(4, DMT):
                    nc.tensor.matmul(
                        out=ps[:, 0:SC],
                        lhsT=wsb[:, a, c * 128:(c + 1) * 128],
                        rhs=xs[:, a, :],
                        start=False, stop=(a == DMT - 1),
                    )
                s0, s1 = sc * SC, (sc + 1) * SC
                # head 2c native rows 0-63; head 2c+1 native rows 64-127
                nc.vector.tensor_scalar_add(
                    out=dsts[0][0:DK, s0:s1], in0=ps[0:DK, 0:SC],
                    scalar1=bsb[0:DK, c:c + 1])
                nc.vector.tensor_scalar_add(
                    out=dsts[1][DK:128, s0:s1], in0=ps[DK:128, 0:SC],
                    scalar1=bsb[DK:128, c:c + 1])
                # duplicate this slice into the other half right away
                # (SBUF->SBUF DMA) so scores kt for this s-range unblock
                nc.sync.dma_start(out=dsts[0][DK:128, s0:s1],
                                  in_=dsts[0][0:DK, s0:s1])
                nc.sync.dma_start(out=dsts[1][0:DK, s0:s1],
                                  in_=dsts[1][DK:128, s0:s1])
            return [emit_a, emit_b]

        # ---- output projection for one 128-row q tile, two 4-MM chunks ----
        def fin_chunks(qt_i):
            state = {}

            def emit_a():
                ps = p_ps.tile([128, 1024], f32, tag="ps", name=f"pso{qt_i}")
                for cc in range(2):
                    for half in range(2):
                        nc.tensor.matmul(
                            out=ps[:, half * 512:(half + 1) * 512],
                            lhsT=ot_sb[cc][:, qt_i * 128:(qt_i + 1) * 128],
                            rhs=wo_sb[:, cc, half * 512:(half + 1) * 512],
                            start=(cc == 0), stop=False,
                        )
                state["ps"] = ps

            def emit_b():
                ps = state["ps"]
                for cc in range(2, NCH):
                    for half in range(2):
                        nc.tensor.matmul(
                            out=ps[:, half * 512:(half + 1) * 512],
                            lhsT=ot_sb[cc][:, qt_i * 128:(qt_i + 1) * 128],
                            rhs=wo_sb[:, cc, half * 512:(half + 1) * 512],
                            start=False, stop=(cc == NCH - 1),
                        )
                st = p_st.tile([128, D], f32, tag="st", name=f"st{qt_i}")
                nc.vector.tensor_copy(out=st[:], in_=ps[:])
                nc.sync.dma_start(out=out[qt_i * 128:(qt_i + 1) * 128, :],
                                  in_=st[:])
            return [emit_a, emit_b]

        # ---- one attention loop: head hg = 2c+hh, q columns qc*QC.. ----
        def attention_loop(c, qc, hh, slots=(), fillers=None, queue=None):
            """fillers: per-loop whole-group callbacks popped at `slots`.
            queue: global 4-MM chunk queue popped at every odd kt."""
            slots = set(slots)
            fillers = fillers if fillers is not None else []
            hg = 2 * c + hh
            if ot_sb[c] is None:
                ot_sb[c] = p_ot.tile([128, S], dmm, tag="ot", name=f"ot{c}")
            pv_ps = p_pv.tile([65, QC], f32, tag="pv",
                              name=f"pv{c}_{qc}_{hh}")
            for kt_i in range(KT):
                if kt_i in slots and fillers:
                    fillers.pop(0)()
                elif queue and kt_i % 2 == 1 and kt_i <= 13:
                    queue.pop(0)()
                rg = DK * (kt_i % 2)
                ps = p_ps.tile([128, QC], f32, tag="ps",
                               name=f"pss{c}_{qc}_{kt_i}_{hh}")
                for half in range(QC // 512):
                    q0 = qc * QC + half * 512
                    nc.tensor.matmul(
                        out=ps[:, half * 512:(half + 1) * 512],
                        lhsT=kt_sb[hg][rg:rg + DK,
                                       kt_i * 128:(kt_i + 1) * 128],
                        rhs=qt_sb[hg][rg:rg + DK, q0:q0 + 512],
                        start=True, stop=True,
                    )
                pt = p_pt.tile([128, QC], dmm, tag="pt",
                               name=f"pt{c}_{qc}_{kt_i}_{hh}")
                nc.scalar.activation(pt[:], ps[:], Exp, bias=0.0, scale=0.125)
                for half in range(QC // 512):
                    nc.tensor.matmul(
                        out=pv_ps[:, half * 512:(half + 1) * 512],
                        lhsT=v_sb[kt_i][:, hg, :],
                        rhs=pt[:, half * 512:(half + 1) * 512],
                        start=(kt_i == 0), stop=(kt_i == KT - 1),
                    )
            # evict PV psum right away to release its bank pair
            ovt = p_ov.tile([65, QC], f32, tag="ov", name=f"ov{c}_{qc}_{hh}")
            nc.vector.tensor_copy(out=ovt[:], in_=pv_ps[:])
            # normalize off the critical path:
            # O^T = PV[0:64] * broadcast(1 / PV[64])
            zs = p_zr.tile([DK, QC // DK], f32, tag="zs",
                           name=f"zs{c}_{qc}_{hh}")
            nc.sync.dma_start(out=zs[:], in_=ovt[DK:DK + 1, :])
            nc.vector.reciprocal(out=zs[:], in_=zs[:])
            zr = p_zr.tile([1, QC], f32, tag="zr", name=f"zr{c}_{qc}_{hh}")
            nc.sync.dma_start(out=zr[:], in_=zs[:])
            rb = p_rb.tile([DK, QC], f32, tag="rb", name=f"rb{c}_{qc}_{hh}")
            nc.gpsimd.partition_broadcast(rb[:], zr[:], channels=DK)
            if hh == 0:
                nc.vector.tensor_mul(
                    out=ot_sb[c][0:DK, qc * QC:(qc + 1) * QC],
                    in0=ovt[0:DK, :], in1=rb[:])
            else:
                tmp = p_rb.tile([DK, QC], dmm, tag="rb", name=f"tmp{c}_{qc}")
                nc.vector.tensor_mul(out=tmp[:], in0=ovt[0:DK, :], in1=rb[:])
                nc.sync.dma_start(
                    out=ot_sb[c][DK:128, qc * QC:(qc + 1) * QC],
                    in_=tmp[:])

        # ================= emission =================
        # prologue: pair-0 Q/K s-chunks 0/1 + V chunk 0; queue order makes
        # the first-needed transfers (wk, xs k0) land first
        alloc_pair(0)
        for e in proj_chunks(0, "k", 0, dma=nc.sync):
            e()
        for e in proj_chunks(0, "q", 0, dma=nc.sync):
            e()
        for e in proj_chunks(0, "q", 1, dma=nc.scalar):
            e()
        for st in range(4):
            emit_v_st(st)
        wo_sb = p_wvo.tile([128, NCH, D], dmm, tag="wo", name="wo_sb")
        nc.scalar.dma_start(out=wo_sb[:], in_=wo)

        alloc_pair(1)
        alloc_pair(2)
        alloc_pair(3)

        # loop (0,0,0) consumes v4..v15 + k1..k3 just-in-time as whole
        # groups (it is PE-bound regardless); everything downstream drains
        # from the global 4-MM chunk queue, one chunk per odd kt
        k01 = proj_chunks(0, "k", 1)
        k02 = proj_chunks(0, "k", 2)
        k03 = proj_chunks(0, "k", 3)
        f000 = [v_filler(4), lambda: (k01[0](), k01[1]()), v_filler(5),
                lambda: (k02[0](), k02[1]()), v_filler(6), v_filler(7),
                lambda: (k03[0](), k03[1]()), v_filler(8), v_filler(9),
                v_filler(10), v_filler(11), v_filler(12), v_filler(13),
                v_filler(14), v_filler(15)]

        FQ = []
        for cc, w, sc in [(0, "q", 2), (0, "q", 3),
                          (1, "k", 0), (1, "q", 0), (1, "q", 1), (1, "k", 1),
                          (1, "k", 2), (1, "q", 2), (1, "k", 3), (1, "q", 3),
                          (2, "k", 0), (2, "q", 0), (2, "q", 1), (2, "k", 1),
                          (2, "k", 2), (2, "q", 2), (2, "k", 3), (2, "q", 3),
                          (3, "k", 0), (3, "q", 0), (3, "q", 1), (3, "k", 1),
                          (3, "k", 2), (3, "q", 2), (3, "k", 3), (3, "q", 3)]:
            FQ.extend(proj_chunks(cc, w, sc))

        FIN = []
        for qt_i in range(8):
            FIN.extend(fin_chunks(qt_i))

        for c in range(NCH):
            for qc in range(NQC):
                for hh in range(2):
                    if (c, qc, hh) == (0, 0, 0):
                        attention_loop(c, qc, hh, slots=range(0, 15),
                                       fillers=f000)
                    elif c == 3 and qc == 1:
                        attention_loop(c, qc, hh, queue=FIN)
                    else:
                        attention_loop(c, qc, hh, queue=FQ)
        # whatever is left: late fin chunks + q tiles 8..15
        for e in FIN:
            e()
        for qt_i in range(8, KT):
            a, b2 = fin_chunks(qt_i)
            a()
            b2()

    nc.compile()
    return nc


def get_program():
    if "nc" not in _CACHE:
        _CACHE["nc"] = _build_program()
    return _CACHE["nc"]


def make_in_maps(inputs):
    dt = _np_mm_dtype()
    q = np.asarray(inputs["query"], np.float32)
    k = np.asarray(inputs["key"], np.float32)
    v = np.asarray(inputs["value"], np.float32)
    Wq = np.asarray(inputs["Wq"], np.float32)
    Wk = np.asarray(inputs["Wk"], np.float32)
    Wv = np.asarray(inputs["Wv"], np.float32)
    Wo = np.asarray(inputs["Wo"], np.float32)
    bq = np.asarray(inputs["bq"], np.float32)
    bk = np.asarray(inputs["bk"], np.float32)

    def slab(x):
        # [S, D] -> x.T [D, S] -> [sc, p, a, s] contiguous slabs
        return np.ascontiguousarray(
            x.T.reshape(DMT, 128, NSC, SC).transpose(2, 1, 0, 3))

    def wtile(W):
        # [D, GD_slice] -> [p, a, d]
        return np.ascontiguousarray(W.reshape(DMT, 128, -1).transpose(1, 0, 2))

    in_maps = []
    for core in range(NCORES):
        b, g = core // 2, core % 2
        sl = slice(g * GD, (g + 1) * GD)
        in_maps.append({
            "xq": slab(q[b]).astype(dt),
            "xk": slab(k[b]).astype(dt),
            "xv": slab(v[b]).astype(dt),
            "wq": wtile(Wq[:, sl]).astype(dt),
            "wk": wtile(Wk[:, sl]).astype(dt),
            "wv": wtile(Wv[:, sl]).astype(dt),
            "wo": np.ascontiguousarray(
                Wo[sl, :].reshape(NCH, 128, D).transpose(1, 0, 2)).astype(dt),
            "bq": np.ascontiguousarray(bq[sl].reshape(NCH, 128).T),
            "bk": np.ascontiguousarray(bk[sl].reshape(NCH, 128).T),
        })
    return in_maps


def combine_outputs(results, inputs):
    Wo = np.asarray(inputs["Wo"], np.float32)
    bv = np.asarray(inputs["bv"], np.float32)
    bo = np.asarray(inputs["bo"], np.float32)
    out = np.empty((B, S, D), np.float32)
    for b in range(B):
        out[b] = results[2 * b]["out"] + results[2 * b + 1]["out"]
    out += bv @ Wo + bo
    return out


def kernel(**inputs):
    from concourse.bass_utils import run_bass_kernel_spmd
    nc = get_program()
    in_maps = make_in_maps(inputs)
    res = run_bass_kernel_spmd(nc, in_maps, list(range(NCORES)))
    return combine_outputs(res.results, inputs)
